# revision 8
# baseline (speedup 1.0000x reference)
"""CrossEfficientAttention on 8 Trainium2 NeuronCores.

Batch-parallel sharding: n=8 batch items, one per core (no collectives).

Per-core math (item x_q, x_k, x_v : [256, 6400]):
    q  = Wq x_q + bq ; k = Wk x_k (+bk cancels over the l-softmax) ; v = Wv x_v + bv
    k_sm = softmax_l(k); q_sm = softmax_ch/head(q)
    ctx  = k_sm @ v^T (per head, 32x32); out = Wr @ (ctx^T @ q_sm) + br + x_q

Numerics: the attention term is ~2% of the output magnitude (the residual
dominates), so the whole attention path runs in fp8e4 (DoubleRow matmuls,
2x PE) while the residual + biases ride at bf16. Output is bf16.
exp(k) is computed as exp(k-2) and exp(q) as exp(q+bq-2) so fp8 never
overflows (the softmax normalizations absorb constant shifts exactly).

Structure (one fused streaming loop + tiny boundary + output pass):
  Pass 1 (per 512-wide l-chunk): stream x_k/x_v as fp8 in [cin-half, 2, l]
    layout, project via one DoubleRow matmul per 128-l block (contraction
    256 in one go), exp(k)-2 on ACT -> ksmT fp8, copy v -> vT fp8; per
    lc-pair one DoubleRow full Gram matmul per cin-half accumulates
    ctx[256ch, 256v] plus an S_k column via a ones rhs; interleaved q
    pipeline (bf16): project, exp(q+bq), per-head sums via BONES matmul,
    fast reciprocal, broadcast back via IND8 matmul, normalize on GPSIMD.
  Boundary: ctx rows scaled by 1/S_k, 4 PE transposes, bv folded into the
    tiny A^T = ctxT^T . Wr^T (bf16) matrix.
  Pass 2: out = (A^T)^T . q_sm + br + x_q via one matmul pair + one
    scalar_tensor_tensor, stream out as bf16.
"""

from contextlib import ExitStack

import ml_dtypes
import numpy as np

import concourse.bacc as bacc
import concourse.bass as bass
import concourse.tile as tile
from concourse import mybir
from concourse.bass_utils import run_bass_kernel_spmd

F32 = mybir.dt.float32
BF16 = mybir.dt.bfloat16
FP8 = mybir.dt.float8e4
EXP = mybir.ActivationFunctionType.Exp
MULT = mybir.AluOpType.mult
ADD = mybir.AluOpType.add
DR = mybir.MatmulPerfMode.DoubleRow

N_CORES = 8
N, CIN, H_IMG, W_IMG = 8, 256, 80, 80
L = H_IMG * W_IMG            # 6400
HEADS = 8
NL128 = L // 128             # 50 l-chunks of 128
NPAIR = NL128 // 2           # 25 lc pairs
LW = 512                     # streaming l tile width
NLW = (L + LW - 1) // LW     # 13 (12x512 + 1x256)

CBB_COLS = 1296              # bf16 pack: wq|wr|pad
CF_COLS = 135                # f32 pack: bq|bv|br|ident|neg2


def _emit(tc: tile.TileContext, ins: dict, out_ap: bass.AP):
    nc = tc.nc
    es = ExitStack()

    # ---------------- persistent consts ----------------
    cpool = es.enter_context(tc.tile_pool(name="consts", bufs=1))
    wk3 = cpool.tile([128, 2, 256], FP8, name="wk3")
    wv3 = cpool.tile([128, 2, 256], FP8, name="wv3")
    wq3 = cpool.tile([128, 2, 256], FP8, name="wq3")
    ones3 = cpool.tile([128, 2, 1], FP8, name="ones3")
    indsel = cpool.tile([128, 4, 256], FP8, name="indsel")
    cbb = cpool.tile([128, CBB_COLS], BF16, name="cbb")
    cf = cpool.tile([128, CF_COLS], F32, name="cf")
    WR = cbb[:, 512:1024]
    BONES = cbb[:, 1024:1040]
    BQ = cf[:, 0:2]
    BV = cf[:, 2:4]
    BR = cf[:, 4:6]
    IDENT = cf[:, 6:134]
    NEG2 = cf[:, 134:135]

    at8 = cpool.tile([128, 2, 256], FP8, name="at8")
    eq83 = cpool.tile([128, 2, L], FP8, name="eq83")
    xqb_sb = [cpool.tile([128, L], BF16, name=f"xqb{k}") for k in range(2)]
    xq83 = cpool.tile([128, 2, L], FP8, name="xq83")

    xqb_ap, xq8_ap = ins["xqb"], ins["xq8"]
    xk_ap, xv_ap = ins["xk8"], ins["xv8"]

    # consts first so WK/WV are resident before the first projection
    nc.sync.dma_start(wk3[:], ins["wk8"][:])
    nc.sync.dma_start(wv3[:], ins["wv8"][:])
    nc.sync.dma_start(wq3[:], ins["wq8"][:])
    nc.sync.dma_start(ones3[:], ins["ones8"][:])
    nc.sync.dma_start(indsel[:], ins["indsel"][:])
    nc.sync.dma_start(cbb[:], ins["cbb"][:])
    nc.sync.dma_start(cf[:], ins["cf"][:])

    # ================= pools =================
    es_a = ExitStack()
    kvpool = es_a.enter_context(tc.tile_pool(name="kv", bufs=2))
    equpool = es_a.enter_context(tc.tile_pool(name="equ", bufs=6))
    rtpool = es_a.enter_context(tc.tile_pool(name="rt", bufs=2))
    bpool = es_a.enter_context(tc.tile_pool(name="bnd", bufs=1))
    # PSUM pools, opened in reverse release order (stack allocator):
    # pq/ps/prb live through the tail; ctx until the boundary; pkv dies first.
    pq_pool = es_a.enter_context(tc.tile_pool(name="pq", bufs=2, space="PSUM"))
    ps_pool = es_a.enter_context(tc.tile_pool(name="ps", bufs=1, space="PSUM"))
    prb_pool = es_a.enter_context(tc.tile_pool(name="prb", bufs=1, space="PSUM"))
    es_ctx = ExitStack()
    bigpool = es_ctx.enter_context(tc.tile_pool(name="big", bufs=1))
    ctxpool = es_ctx.enter_context(tc.tile_pool(name="ctxp", bufs=1, space="PSUM"))
    es_kv = ExitStack()
    pkv = es_kv.enter_context(tc.tile_pool(name="pkv", bufs=2, space="PSUM"))

    ksmT = bigpool.tile([128, NL128, 256], FP8, name="ksmT")
    vT = bigpool.tile([128, NL128, 256], FP8, name="vT")

    ctx_ps = [ctxpool.tile([128, 257], F32, name=f"ctx{c}") for c in range(2)]

    def gram_pair(pp):
        # full [256ch, 256v] Gram + S_k column, one DoubleRow matmul per
        # cin-half, contracting both lc chunks of the pair at once
        for c in range(2):
            nc.tensor.matmul(
                ctx_ps[c][:, 0:256],
                ksmT[:, 2 * pp : 2 * pp + 2, 128 * c : 128 * c + 128],
                vT[:, 2 * pp : 2 * pp + 2, :],
                start=(pp == 0), stop=(pp == NPAIR - 1),
                perf_mode=DR,
            )
            nc.tensor.matmul(
                ctx_ps[c][:, 256:257],
                ksmT[:, 2 * pp : 2 * pp + 2, 128 * c : 128 * c + 128],
                ones3[:, 0:2, :],
                start=(pp == 0), stop=(pp == NPAIR - 1),
                perf_mode=DR,
            )

    equ_tiles = {}

    def qwork(a):
        # q projection (DoubleRow) + exp -> fp8 scratch equ [128, 2, w]
        w = min(LW, L - a * LW)
        l0 = a * LW
        equ = equpool.tile([128, 2, w], FP8, name="equ")
        for c in range(2):
            pq = pq_pool.tile([128, w], F32, name="pq")
            nc.tensor.matmul(
                pq[:],
                wq3[:, 0:2, 128 * c : 128 * c + 128],
                xq83[:, 0:2, l0 : l0 + w],
                start=True, stop=True, perf_mode=DR,
            )
            # exp(q + bq - 2): fp8-safe; the ch-softmax absorbs e^-2
            nc.scalar.activation(equ[:, c, :], pq[:], EXP, bias=BQ[:, c : c + 1])
        equ_tiles[a] = equ

    psS4 = {}
    rtb4 = {}

    def qsum(a):
        # per-head sums for chunk a, packed 4 chunks per [128, 512] psum tile
        w = min(LW, L - a * LW)
        g, d = a // 4, a % 4
        if d == 0:
            psS4[g] = ps_pool.tile([128, LW], F32, name="psS4")
            # unused rows feed the full-128 indsel contraction: keep finite
            nc.vector.memset(psS4[g][:], 1.0)
        for c in range(2):
            nc.tensor.matmul(
                psS4[g][32 * d : 32 * d + 8, 0:w],
                BONES[:, 8 * c : 8 * c + 8],
                equ_tiles[a][:, c, :],
                start=(c == 0), stop=(c == 1),
                tile_position=(0, 32 * d),
            )

    def qrecip(g):
        # one reciprocal + cast for a whole group of 4 chunks
        ps = psS4.pop(g)
        rt = rtpool.tile([128, LW], F32, name="rt")
        rtb = rtpool.tile([128, LW], BF16, name="rtb")
        nc.vector.reciprocal_approx_fast(rt[:], ps[:])
        nc.vector.tensor_copy(rtb[:], rt[:])
        rtb4[g] = rtb

    def qnorm(a, c):
        # broadcast 1/S to all head partitions (PE), stage to SBUF bf16,
        # then normalize on the otherwise-idle GPSIMD -> eq83 fp8
        w = min(LW, L - a * LW)
        l0 = a * LW
        g, d = a // 4, a % 4
        rtb = rtb4[g]
        prb = prb_pool.tile([128, w], F32, name="prb")
        nc.tensor.matmul(
            prb[:], indsel[:, d, 128 * c : 128 * c + 128], rtb[:, 0:w],
            start=True, stop=True,
        )
        equ = equ_tiles[a] if c == 0 else equ_tiles.pop(a)
        prbs = kvpool.tile([128, w], BF16, name="prbs")
        if (2 * a + c) % 4 == 1:
            nc.vector.tensor_copy(prbs[:], prb[:])
        else:
            nc.scalar.copy(prbs[:], prb[:])
        nc.gpsimd.tensor_tensor(
            eq83[:, c, l0 : l0 + w], equ[:, c, :], prbs[:], op=MULT
        )

    qn = [0]  # count of emitted qnorm half-steps (2 per chunk, in order)

    def qstages(t):
        if 0 <= t - 2 < NLW:
            qwork(t - 2)
        if 0 <= t - 3 < NLW:
            a = t - 3
            qsum(a)
            if a % 4 == 3 or a == NLW - 1:
                qrecip(a // 4)
        # emit up to 2 qnorm half-steps whose group reciprocal is ready
        steps = 0
        while steps < 2 and qn[0] < 2 * NLW:
            a, c = qn[0] // 2, qn[0] % 2
            if a // 4 not in rtb4 or a > t - 4:
                break
            qnorm(a, c)
            qn[0] += 1
            steps += 1

    # ================= pass 1: k/v proj + Gram with interleaved q =================
    xk_t = xv_t = None
    for a in range(NLW):
        w = min(LW, L - a * LW)
        l0 = a * LW
        if a % 2 == 0:
            # 1024-wide fp8 loads (2 chunks worth); bf16 q ships alongside so
            # the q pipeline starts early
            wd = min(2 * LW, L - l0)
            xk_t = kvpool.tile([128, 2, wd], FP8, name="xk3")
            xv_t = kvpool.tile([128, 2, wd], FP8, name="xv3")
            for k in range(2):
                nc.sync.dma_start(xk_t[:, k, :], xk_ap[128 * k : 128 * (k + 1), l0 : l0 + wd])
                nc.sync.dma_start(xv_t[:, k, :], xv_ap[128 * k : 128 * (k + 1), l0 : l0 + wd])
            for k in range(2):
                nc.sync.dma_start(
                    xq83[:, k, l0 : l0 + wd], xq8_ap[128 * k : 128 * (k + 1), l0 : l0 + wd]
                )
                nc.sync.dma_start(
                    xqb_sb[k][:, l0 : l0 + wd], xqb_ap[128 * k : 128 * (k + 1), l0 : l0 + wd]
                )
        off = 512 * (a % 2)
        for jj in range(w // 256):  # one lc pair per psum tile
            pp = a * 2 + jj
            lc = 2 * pp
            pk = pkv.tile([128, 512], F32, name="pkv")
            pv = pkv.tile([128, 512], F32, name="pkv")
            for j in range(2):
                o = off + 256 * jj + 128 * j
                nc.tensor.matmul(
                    pk[:, 256 * j : 256 * j + 256],
                    xk_t[:, 0:2, o : o + 128], wk3[:, 0:2, :],
                    start=True, stop=True, perf_mode=DR,
                )
                nc.tensor.matmul(
                    pv[:, 256 * j : 256 * j + 256],
                    xv_t[:, 0:2, o : o + 128], wv3[:, 0:2, :],
                    start=True, stop=True, perf_mode=DR,
                )
            # exp(k-2): keeps fp8 in range; 1/S_k normalization absorbs e^-2
            nc.scalar.activation(ksmT[:, lc : lc + 2, :], pk[:], EXP, bias=NEG2[:, 0:1])
            if pp % 4 == 1:
                nc.scalar.copy(vT[:, lc : lc + 2, :], pv[:])
            else:
                nc.vector.tensor_copy(vT[:, lc : lc + 2, :], pv[:])
            if pp - 2 >= 0:
                gram_pair(pp - 2)
        qstages(a)

    for pp in range(NPAIR - 2, NPAIR):
        gram_pair(pp)

    es_kv.close()  # release pk/pv banks for the boundary

    # ---------------- boundary: build A^T [hk, c] (bf16) ----------------
    es_bnd = ExitStack()
    bpsum = es_bnd.enter_context(tc.tile_pool(name="bndp", bufs=2, space="PSUM"))
    rk = [bpool.tile([128, 1], F32, name=f"rk{c}") for c in range(2)]
    ctxs = [bpool.tile([128, 256], F32, name=f"ctxs{c}") for c in range(2)]
    for c in range(2):
        nc.vector.reciprocal(rk[c][:], ctx_ps[c][:, 256:257])
        nc.vector.tensor_scalar_mul(ctxs[c][:], ctx_ps[c][:, 0:256], rk[c][:])
    ctxT_ps = [bpsum.tile([128, 256], F32, name="bnd") for a in range(2)]
    for a in range(2):
        for b in range(2):
            nc.tensor.transpose(
                ctxT_ps[a][:, 128 * b : 128 * b + 128],
                ctxs[b][:, 128 * a : 128 * a + 128],
                IDENT,
            )
    ctxT_sb = [bpool.tile([128, 256], BF16, name=f"ctxTs{a}") for a in range(2)]
    for a in range(2):
        nc.vector.memset(ctxT_sb[a][:], 0.0)
    for h in range(HEADS):
        a = h // 4
        p = 32 * (h % 4)
        nc.vector.tensor_scalar_add(
            ctxT_sb[a][p : p + 32, 32 * h : 32 * h + 32],
            ctxT_ps[a][p : p + 32, 32 * h : 32 * h + 32],
            BV[p : p + 32, a : a + 1],
        )
    at_ps = [bpsum.tile([128, 256], F32, name="bnd") for g in range(2)]
    for g in range(2):
        for a in range(2):
            nc.tensor.matmul(
                at_ps[g][:],
                ctxT_sb[a][:, 128 * g : 128 * g + 128],
                WR[:, 256 * a : 256 * a + 256],
                start=(a == 0), stop=(a == 1),
            )
        if g == 0:
            nc.scalar.copy(at8[:, g, :], at_ps[g][:])
        else:
            nc.vector.tensor_copy(at8[:, g, :], at_ps[g][:])
    es_bnd.close()
    es_ctx.close()

    # ========= tail: remaining q stages interleaved with pass-2 output =========
    es_c = ExitStack()
    opool = es_c.enter_context(tc.tile_pool(name="op", bufs=3))
    po_pool = es_c.enter_context(tc.tile_pool(name="po", bufs=4, space="PSUM"))

    def pass2(a):
        wd = min(2 * LW, L - a * LW)
        ld = a * LW
        for c in range(2):
            ob = opool.tile([128, wd], BF16, name="ob")
            for half in range(0, wd, LW):
                w = min(LW, wd - half)
                l0 = ld + half
                po = po_pool.tile([128, w], F32, name="po")
                nc.tensor.matmul(
                    po[:], at8[:, 0:2, 128 * c : 128 * c + 128],
                    eq83[:, 0:2, l0 : l0 + w],
                    start=True, stop=True, perf_mode=DR,
                )
                nc.vector.scalar_tensor_tensor(
                    ob[:, half : half + w], po[:], BR[:, c : c + 1],
                    xqb_sb[c][:, l0 : l0 + w], op0=ADD, op1=ADD,
                )
            nc.sync.dma_start(out_ap[128 * c : 128 * c + 128, ld : ld + wd], ob[:])

    p2 = 0  # next pass-2 pair start chunk
    for t in range(NLW, NLW + 9):
        qstages(t)
        # pair (p2, p2+1) ready once both chunks are fully normalized
        while p2 < NLW and 2 * min(p2 + 2, NLW) <= qn[0]:
            pass2(p2)
            p2 += 2
    es_c.close()
    es_a.close()
    es.close()


def _build_consts(Wq, bq, Wk, bk, Wv, bv, Wr, br):
    bf = ml_dtypes.bfloat16
    f8 = ml_dtypes.float8_e4m3

    def packT(Wt):  # [cout, cin] -> [128, 512], col block k = W.T[128k:128k+128, :]
        t = np.ascontiguousarray(np.asarray(Wt, np.float32).T)
        return np.concatenate([t[0:128, :], t[128:256, :]], axis=1)

    def pack3(Wt):  # [cout, cin] -> [128, 2, 256] DoubleRow stationary layout
        t = np.ascontiguousarray(np.asarray(Wt, np.float32).T)  # [cin, cout]
        return np.ascontiguousarray(
            t.reshape(2, 128, 256).transpose(1, 0, 2)
        ).astype(f8)

    f8c = ml_dtypes.float8_e4m3
    ch = np.arange(256)
    bones_full = (ch[:, None] // 32 == np.arange(8)[None, :]).astype(np.float32)  # [256, 8]
    bones = np.concatenate([bones_full[0:128, :], bones_full[128:256, :]], axis=1)
    # indsel[p, d, ch] = 1 iff p == 32d + head-in-group(ch): broadcasts the
    # packed [128, 512] reciprocal tile rows back to all head partitions
    indsel = np.zeros((128, 4, 256), np.float32)
    for d in range(4):
        for chv in range(256):
            indsel[32 * d + chv // 32 % 8, d, chv] = 1.0
    cbb = np.concatenate(
        [packT(Wq), packT(Wr), bones, np.zeros((128, 256), np.float32)], axis=1
    ).astype(bf)
    assert cbb.shape == (128, CBB_COLS), cbb.shape

    def two(v):
        return np.stack([v[0:128], v[128:256]], axis=1).astype(np.float32)

    cf = np.concatenate(
        [two(np.asarray(bq) - 2.0), two(np.asarray(bv)), two(np.asarray(br)),
         np.eye(128, dtype=np.float32), np.full((128, 1), -2.0, np.float32)], axis=1
    ).astype(np.float32)
    assert cf.shape == (128, CF_COLS), cf.shape
    return {
        "wk8": pack3(Wk), "wv8": pack3(Wv), "wq8": pack3(Wq),
        "ones8": np.ones((128, 2, 1), np.float32).astype(f8),
        "indsel": indsel.astype(f8c),
        "cbb": cbb, "cf": cf,
    }


_NC = None


def _build():
    nc = bacc.Bacc("TRN2", target_bir_lowering=False)
    ins = {}
    ins["xqb"] = nc.dram_tensor("xqb", [CIN, L], BF16, kind="ExternalInput").ap()
    ins["xq8"] = nc.dram_tensor("xq8", [CIN, L], FP8, kind="ExternalInput").ap()
    ins["xk8"] = nc.dram_tensor("xk8", [CIN, L], FP8, kind="ExternalInput").ap()
    ins["xv8"] = nc.dram_tensor("xv8", [CIN, L], FP8, kind="ExternalInput").ap()
    ins["wk8"] = nc.dram_tensor("wk8", [128, 2, 256], FP8, kind="ExternalInput").ap()
    ins["wv8"] = nc.dram_tensor("wv8", [128, 2, 256], FP8, kind="ExternalInput").ap()
    ins["wq8"] = nc.dram_tensor("wq8", [128, 2, 256], FP8, kind="ExternalInput").ap()
    ins["ones8"] = nc.dram_tensor("ones8", [128, 2, 1], FP8, kind="ExternalInput").ap()
    ins["indsel"] = nc.dram_tensor("indsel", [128, 4, 256], FP8, kind="ExternalInput").ap()
    ins["cbb"] = nc.dram_tensor("cbb", [128, CBB_COLS], BF16, kind="ExternalInput").ap()
    ins["cf"] = nc.dram_tensor("cf", [128, CF_COLS], F32, kind="ExternalInput").ap()
    out_ap = nc.dram_tensor("out", [CIN, L], BF16, kind="ExternalOutput").ap()
    with tile.TileContext(nc) as tc:
        _emit(tc, ins, out_ap)
    nc.compile()
    return nc


def get_nc():
    global _NC
    if _NC is None:
        _NC = _build()
    return _NC


def make_in_maps(inputs):
    bf = ml_dtypes.bfloat16
    f8 = ml_dtypes.float8_e4m3
    consts = _build_consts(
        inputs["Wq"], inputs["bq"], inputs["Wk"], inputs["bk"],
        inputs["Wv"], inputs["bv"], inputs["Wr"], inputs["br"],
    )
    qf32 = np.asarray(inputs["query_feature"], np.float32).reshape(N, CIN, L)
    qf = qf32.astype(bf)
    qf8 = qf32.astype(f8)
    kf = np.asarray(inputs["key_feature"], np.float32).reshape(N, CIN, L).astype(f8)
    vf = np.asarray(inputs["value_feature"], np.float32).reshape(N, CIN, L).astype(f8)
    return [
        {"xqb": np.ascontiguousarray(qf[i]),
         "xq8": np.ascontiguousarray(qf8[i]),
         "xk8": np.ascontiguousarray(kf[i]),
         "xv8": np.ascontiguousarray(vf[i]),
         **consts}
        for i in range(N_CORES)
    ]


def kernel(query_feature, key_feature, value_feature,
           Wq, bq, Wk, bk, Wv, bv, Wr, br):
    nc = get_nc()
    in_maps = make_in_maps(dict(
        query_feature=query_feature, key_feature=key_feature,
        value_feature=value_feature, Wq=Wq, bq=bq, Wk=Wk, bk=bk,
        Wv=Wv, bv=bv, Wr=Wr, br=br,
    ))
    res = run_bass_kernel_spmd(nc, in_maps, core_ids=list(range(N_CORES)))
    out = np.stack(
        [res.results[i]["out"].astype(np.float32) for i in range(N_CORES)]
    )
    return out.reshape(N, CIN, H_IMG, W_IMG)


# revision 9
# speedup vs baseline: 1.0261x; 1.0261x over previous
"""CrossEfficientAttention on 8 Trainium2 NeuronCores.

Batch-parallel sharding: n=8 batch items, one per core (no collectives).

Per-core math (item x_q, x_k, x_v : [256, 6400]):
    q  = Wq x_q + bq ; k = Wk x_k (+bk cancels over the l-softmax) ; v = Wv x_v + bv
    k_sm = softmax_l(k); q_sm = softmax_ch/head(q)
    ctx  = k_sm @ v^T (per head, 32x32); out = Wr @ (ctx^T @ q_sm) + br + x_q

Numerics: the attention term is ~2% of the output magnitude (the residual
dominates), so the whole attention path runs in fp8e4 (DoubleRow matmuls,
2x PE) while the residual + biases ride at bf16. Output is bf16.
exp(k) is computed as exp(k-2) and exp(q) as exp(q+bq-2) so fp8 never
overflows (the softmax normalizations absorb constant shifts exactly).

Structure (one fused streaming loop + tiny boundary + output pass):
  Pass 1 (per 512-wide l-chunk): stream x_k/x_v as fp8 in [cin-half, 2, l]
    layout, project via one DoubleRow matmul per 128-l block (contraction
    256 in one go), exp(k)-2 on ACT -> ksmT fp8, copy v -> vT fp8; per
    lc-pair one DoubleRow full Gram matmul per cin-half accumulates
    ctx[256ch, 256v] plus an S_k column via a ones rhs; interleaved q
    pipeline (bf16): project, exp(q+bq), per-head sums via BONES matmul,
    fast reciprocal, broadcast back via IND8 matmul, normalize on GPSIMD.
  Boundary: ctx rows scaled by 1/S_k, 4 PE transposes, bv folded into the
    tiny A^T = ctxT^T . Wr^T (bf16) matrix.
  Pass 2: out = (A^T)^T . q_sm + br + x_q via one matmul pair + one
    scalar_tensor_tensor, stream out as bf16.
"""

from contextlib import ExitStack

import ml_dtypes
import numpy as np

import concourse.bacc as bacc
import concourse.bass as bass
import concourse.tile as tile
from concourse import mybir
from concourse.bass_utils import run_bass_kernel_spmd

F32 = mybir.dt.float32
BF16 = mybir.dt.bfloat16
FP8 = mybir.dt.float8e4
EXP = mybir.ActivationFunctionType.Exp
MULT = mybir.AluOpType.mult
ADD = mybir.AluOpType.add
DR = mybir.MatmulPerfMode.DoubleRow

N_CORES = 8
N, CIN, H_IMG, W_IMG = 8, 256, 80, 80
L = H_IMG * W_IMG            # 6400
HEADS = 8
NL128 = L // 128             # 50 l-chunks of 128
NPAIR = NL128 // 2           # 25 lc pairs
LW = 512                     # streaming l tile width
NLW = (L + LW - 1) // LW     # 13 (12x512 + 1x256)

CBB_COLS = 1296              # bf16 pack: wq|wr|pad
CF_COLS = 135                # f32 pack: bq|bv|br|ident|neg2


def _emit(tc: tile.TileContext, ins: dict, out_ap: bass.AP):
    nc = tc.nc
    es = ExitStack()

    # ---------------- persistent consts ----------------
    cpool = es.enter_context(tc.tile_pool(name="consts", bufs=1))
    wk3 = cpool.tile([128, 2, 256], FP8, name="wk3")
    wv3 = cpool.tile([128, 2, 256], FP8, name="wv3")
    wq3 = cpool.tile([128, 2, 256], FP8, name="wq3")
    ones3 = cpool.tile([128, 2, 1], FP8, name="ones3")
    indsel = cpool.tile([128, 4, 256], FP8, name="indsel")
    cbb = cpool.tile([128, CBB_COLS], BF16, name="cbb")
    cf = cpool.tile([128, CF_COLS], F32, name="cf")
    WR = cbb[:, 512:1024]
    BONES = cbb[:, 1024:1040]
    BQ = cf[:, 0:2]
    BV = cf[:, 2:4]
    BR = cf[:, 4:6]
    IDENT = cf[:, 6:134]
    NEG2 = cf[:, 134:135]

    at8 = cpool.tile([128, 2, 256], FP8, name="at8")
    eq83 = cpool.tile([128, 2, L], FP8, name="eq83")
    xqb_sb = [cpool.tile([128, L], BF16, name=f"xqb{k}") for k in range(2)]
    xq83 = cpool.tile([128, 2, L], FP8, name="xq83")

    xqb_ap, xq8_ap = ins["xqb"], ins["xq8"]
    xk_ap, xv_ap = ins["xk8"], ins["xv8"]

    # consts first so WK/WV are resident before the first projection
    nc.sync.dma_start(wk3[:], ins["wk8"][:])
    nc.sync.dma_start(wv3[:], ins["wv8"][:])
    nc.sync.dma_start(wq3[:], ins["wq8"][:])
    nc.sync.dma_start(ones3[:], ins["ones8"][:])
    nc.sync.dma_start(indsel[:], ins["indsel"][:])
    nc.sync.dma_start(cbb[:], ins["cbb"][:])
    nc.sync.dma_start(cf[:], ins["cf"][:])

    # ================= pools =================
    es_a = ExitStack()
    kvpool = es_a.enter_context(tc.tile_pool(name="kv", bufs=2))
    equpool = es_a.enter_context(tc.tile_pool(name="equ", bufs=6))
    rtpool = es_a.enter_context(tc.tile_pool(name="rt", bufs=2))
    bpool = es_a.enter_context(tc.tile_pool(name="bnd", bufs=1))
    # PSUM pools, opened in reverse release order (stack allocator):
    # pq/ps/prb live through the tail; ctx until the boundary; pkv dies first.
    pq_pool = es_a.enter_context(tc.tile_pool(name="pq", bufs=2, space="PSUM"))
    ps_pool = es_a.enter_context(tc.tile_pool(name="ps", bufs=1, space="PSUM"))
    prb_pool = es_a.enter_context(tc.tile_pool(name="prb", bufs=1, space="PSUM"))
    es_ctx = ExitStack()
    bigpool = es_ctx.enter_context(tc.tile_pool(name="big", bufs=1))
    ctxpool = es_ctx.enter_context(tc.tile_pool(name="ctxp", bufs=1, space="PSUM"))
    es_kv = ExitStack()
    pkv = es_kv.enter_context(tc.tile_pool(name="pkv", bufs=2, space="PSUM"))

    ksmT = bigpool.tile([128, NL128, 256], FP8, name="ksmT")
    vT = bigpool.tile([128, NL128, 256], FP8, name="vT")

    ctx_ps = [ctxpool.tile([128, 257], F32, name=f"ctx{c}") for c in range(2)]

    def gram_pair(pp):
        # full [256ch, 256v] Gram + S_k column, one DoubleRow matmul per
        # cin-half, contracting both lc chunks of the pair at once
        for c in range(2):
            nc.tensor.matmul(
                ctx_ps[c][:, 0:256],
                ksmT[:, 2 * pp : 2 * pp + 2, 128 * c : 128 * c + 128],
                vT[:, 2 * pp : 2 * pp + 2, :],
                start=(pp == 0), stop=(pp == NPAIR - 1),
                perf_mode=DR,
            )
            nc.tensor.matmul(
                ctx_ps[c][:, 256:257],
                ksmT[:, 2 * pp : 2 * pp + 2, 128 * c : 128 * c + 128],
                ones3[:, 0:2, :],
                start=(pp == 0), stop=(pp == NPAIR - 1),
                perf_mode=DR,
            )

    equ_tiles = {}

    def qwork(a):
        # q projection (DoubleRow) + exp -> fp8 scratch equ [128, 2, w]
        w = min(LW, L - a * LW)
        l0 = a * LW
        equ = equpool.tile([128, 2, w], FP8, name="equ")
        for c in range(2):
            pq = pq_pool.tile([128, w], F32, name="pq")
            nc.tensor.matmul(
                pq[:],
                wq3[:, 0:2, 128 * c : 128 * c + 128],
                xq83[:, 0:2, l0 : l0 + w],
                start=True, stop=True, perf_mode=DR,
            )
            # exp(q + bq - 2): fp8-safe; the ch-softmax absorbs e^-2
            nc.scalar.activation(equ[:, c, :], pq[:], EXP, bias=BQ[:, c : c + 1])
        equ_tiles[a] = equ

    psS4 = {}
    rtb4 = {}

    def qsum(a):
        # per-head sums for chunk a, packed 4 chunks per [128, 512] psum tile
        w = min(LW, L - a * LW)
        g, d = a // 4, a % 4
        if d == 0:
            psS4[g] = ps_pool.tile([128, LW], F32, name="psS4")
            # unused rows feed the full-128 indsel contraction: keep finite
            nc.vector.memset(psS4[g][:], 1.0)
        for c in range(2):
            nc.tensor.matmul(
                psS4[g][32 * d : 32 * d + 8, 0:w],
                BONES[:, 8 * c : 8 * c + 8],
                equ_tiles[a][:, c, :],
                start=(c == 0), stop=(c == 1),
                tile_position=(0, 32 * d),
            )

    def qrecip(g):
        # one reciprocal + cast for a whole group of 4 chunks
        ps = psS4.pop(g)
        rt = rtpool.tile([128, LW], F32, name="rt")
        rtb = rtpool.tile([128, LW], BF16, name="rtb")
        nc.vector.reciprocal_approx_fast(rt[:], ps[:])
        nc.vector.tensor_copy(rtb[:], rt[:])
        rtb4[g] = rtb

    def qnorm(a, c):
        # broadcast 1/S to all head partitions (PE), stage to SBUF bf16,
        # then normalize on the otherwise-idle GPSIMD -> eq83 fp8
        w = min(LW, L - a * LW)
        l0 = a * LW
        g, d = a // 4, a % 4
        rtb = rtb4[g]
        prb = prb_pool.tile([128, w], F32, name="prb")
        nc.tensor.matmul(
            prb[:], indsel[:, d, 128 * c : 128 * c + 128], rtb[:, 0:w],
            start=True, stop=True,
        )
        equ = equ_tiles[a] if c == 0 else equ_tiles.pop(a)
        prbs = kvpool.tile([128, w], BF16, name="prbs")
        if (2 * a + c) % 4 == 1:
            nc.vector.tensor_copy(prbs[:], prb[:])
        else:
            nc.scalar.copy(prbs[:], prb[:])
        nc.gpsimd.tensor_tensor(
            eq83[:, c, l0 : l0 + w], equ[:, c, :], prbs[:], op=MULT
        )

    qn = [0]  # count of emitted qnorm half-steps (2 per chunk, in order)

    def qstages(t):
        if 0 <= t - 1 < NLW:
            qwork(t - 1)
        if 0 <= t - 2 < NLW:
            a = t - 2
            qsum(a)
            if a % 4 == 3 or a == NLW - 1:
                qrecip(a // 4)
        # emit up to 2 qnorm half-steps whose group reciprocal is ready
        steps = 0
        while steps < 2 and qn[0] < 2 * NLW:
            a, c = qn[0] // 2, qn[0] % 2
            if a // 4 not in rtb4 or a > t - 3:
                break
            qnorm(a, c)
            qn[0] += 1
            steps += 1

    # ================= pass 1: k/v proj + Gram with interleaved q =================
    xk_t = xv_t = None
    for a in range(NLW):
        w = min(LW, L - a * LW)
        l0 = a * LW
        if a % 2 == 0:
            # 1024-wide fp8 loads (2 chunks worth); bf16 q ships alongside so
            # the q pipeline starts early
            wd = min(2 * LW, L - l0)
            xk_t = kvpool.tile([128, 2, wd], FP8, name="xk3")
            xv_t = kvpool.tile([128, 2, wd], FP8, name="xv3")
            for k in range(2):
                nc.sync.dma_start(xk_t[:, k, :], xk_ap[128 * k : 128 * (k + 1), l0 : l0 + wd])
                nc.sync.dma_start(xv_t[:, k, :], xv_ap[128 * k : 128 * (k + 1), l0 : l0 + wd])
            for k in range(2):
                nc.sync.dma_start(
                    xq83[:, k, l0 : l0 + wd], xq8_ap[128 * k : 128 * (k + 1), l0 : l0 + wd]
                )
                nc.sync.dma_start(
                    xqb_sb[k][:, l0 : l0 + wd], xqb_ap[128 * k : 128 * (k + 1), l0 : l0 + wd]
                )
        off = 512 * (a % 2)
        for jj in range(w // 256):  # one lc pair per psum tile
            pp = a * 2 + jj
            lc = 2 * pp
            pk = pkv.tile([128, 512], F32, name="pkv")
            pv = pkv.tile([128, 512], F32, name="pkv")
            for j in range(2):
                o = off + 256 * jj + 128 * j
                nc.tensor.matmul(
                    pk[:, 256 * j : 256 * j + 256],
                    xk_t[:, 0:2, o : o + 128], wk3[:, 0:2, :],
                    start=True, stop=True, perf_mode=DR,
                )
                nc.tensor.matmul(
                    pv[:, 256 * j : 256 * j + 256],
                    xv_t[:, 0:2, o : o + 128], wv3[:, 0:2, :],
                    start=True, stop=True, perf_mode=DR,
                )
            # exp(k-2): keeps fp8 in range; 1/S_k normalization absorbs e^-2
            nc.scalar.activation(ksmT[:, lc : lc + 2, :], pk[:], EXP, bias=NEG2[:, 0:1])
            if pp % 4 == 1:
                nc.scalar.copy(vT[:, lc : lc + 2, :], pv[:])
            else:
                nc.vector.tensor_copy(vT[:, lc : lc + 2, :], pv[:])
            if pp - 2 >= 0:
                gram_pair(pp - 2)
        qstages(a)

    for pp in range(NPAIR - 2, NPAIR):
        gram_pair(pp)

    es_kv.close()  # release pk/pv banks for the boundary

    # ---------------- boundary: build A^T [hk, c] (bf16) ----------------
    es_bnd = ExitStack()
    bpsum = es_bnd.enter_context(tc.tile_pool(name="bndp", bufs=2, space="PSUM"))
    rk = [bpool.tile([128, 1], F32, name=f"rk{c}") for c in range(2)]
    ctxs = [bpool.tile([128, 256], F32, name=f"ctxs{c}") for c in range(2)]
    for c in range(2):
        nc.vector.reciprocal(rk[c][:], ctx_ps[c][:, 256:257])
        nc.vector.tensor_scalar_mul(ctxs[c][:], ctx_ps[c][:, 0:256], rk[c][:])
    ctxT_ps = [bpsum.tile([128, 256], F32, name="bnd") for a in range(2)]
    for a in range(2):
        for b in range(2):
            nc.tensor.transpose(
                ctxT_ps[a][:, 128 * b : 128 * b + 128],
                ctxs[b][:, 128 * a : 128 * a + 128],
                IDENT,
            )
    ctxT_sb = [bpool.tile([128, 256], BF16, name=f"ctxTs{a}") for a in range(2)]
    for a in range(2):
        nc.vector.memset(ctxT_sb[a][:], 0.0)
    for h in range(HEADS):
        a = h // 4
        p = 32 * (h % 4)
        nc.vector.tensor_scalar_add(
            ctxT_sb[a][p : p + 32, 32 * h : 32 * h + 32],
            ctxT_ps[a][p : p + 32, 32 * h : 32 * h + 32],
            BV[p : p + 32, a : a + 1],
        )
    at_ps = [bpsum.tile([128, 256], F32, name="bnd") for g in range(2)]
    for g in range(2):
        for a in range(2):
            nc.tensor.matmul(
                at_ps[g][:],
                ctxT_sb[a][:, 128 * g : 128 * g + 128],
                WR[:, 256 * a : 256 * a + 256],
                start=(a == 0), stop=(a == 1),
            )
        if g == 0:
            nc.scalar.copy(at8[:, g, :], at_ps[g][:])
        else:
            nc.vector.tensor_copy(at8[:, g, :], at_ps[g][:])
    es_bnd.close()
    es_ctx.close()

    # ========= tail: remaining q stages interleaved with pass-2 output =========
    es_c = ExitStack()
    opool = es_c.enter_context(tc.tile_pool(name="op", bufs=3))
    po_pool = es_c.enter_context(tc.tile_pool(name="po", bufs=4, space="PSUM"))

    def pass2(a):
        wd = min(2 * LW, L - a * LW)
        ld = a * LW
        for c in range(2):
            ob = opool.tile([128, wd], BF16, name="ob")
            for half in range(0, wd, LW):
                w = min(LW, wd - half)
                l0 = ld + half
                po = po_pool.tile([128, w], F32, name="po")
                nc.tensor.matmul(
                    po[:], at8[:, 0:2, 128 * c : 128 * c + 128],
                    eq83[:, 0:2, l0 : l0 + w],
                    start=True, stop=True, perf_mode=DR,
                )
                nc.vector.scalar_tensor_tensor(
                    ob[:, half : half + w], po[:], BR[:, c : c + 1],
                    xqb_sb[c][:, l0 : l0 + w], op0=ADD, op1=ADD,
                )
            nc.sync.dma_start(out_ap[128 * c : 128 * c + 128, ld : ld + wd], ob[:])

    p2 = 0  # next pass-2 pair start chunk
    for t in range(NLW, NLW + 9):
        qstages(t)
        # pair (p2, p2+1) ready once both chunks are fully normalized
        while p2 < NLW and 2 * min(p2 + 2, NLW) <= qn[0]:
            pass2(p2)
            p2 += 2
    es_c.close()
    es_a.close()
    es.close()


def _build_consts(Wq, bq, Wk, bk, Wv, bv, Wr, br):
    bf = ml_dtypes.bfloat16
    f8 = ml_dtypes.float8_e4m3

    def packT(Wt):  # [cout, cin] -> [128, 512], col block k = W.T[128k:128k+128, :]
        t = np.ascontiguousarray(np.asarray(Wt, np.float32).T)
        return np.concatenate([t[0:128, :], t[128:256, :]], axis=1)

    def pack3(Wt):  # [cout, cin] -> [128, 2, 256] DoubleRow stationary layout
        t = np.ascontiguousarray(np.asarray(Wt, np.float32).T)  # [cin, cout]
        return np.ascontiguousarray(
            t.reshape(2, 128, 256).transpose(1, 0, 2)
        ).astype(f8)

    f8c = ml_dtypes.float8_e4m3
    ch = np.arange(256)
    bones_full = (ch[:, None] // 32 == np.arange(8)[None, :]).astype(np.float32)  # [256, 8]
    bones = np.concatenate([bones_full[0:128, :], bones_full[128:256, :]], axis=1)
    # indsel[p, d, ch] = 1 iff p == 32d + head-in-group(ch): broadcasts the
    # packed [128, 512] reciprocal tile rows back to all head partitions
    indsel = np.zeros((128, 4, 256), np.float32)
    for d in range(4):
        for chv in range(256):
            indsel[32 * d + chv // 32 % 8, d, chv] = 1.0
    cbb = np.concatenate(
        [packT(Wq), packT(Wr), bones, np.zeros((128, 256), np.float32)], axis=1
    ).astype(bf)
    assert cbb.shape == (128, CBB_COLS), cbb.shape

    def two(v):
        return np.stack([v[0:128], v[128:256]], axis=1).astype(np.float32)

    cf = np.concatenate(
        [two(np.asarray(bq) - 2.0), two(np.asarray(bv)), two(np.asarray(br)),
         np.eye(128, dtype=np.float32), np.full((128, 1), -2.0, np.float32)], axis=1
    ).astype(np.float32)
    assert cf.shape == (128, CF_COLS), cf.shape
    return {
        "wk8": pack3(Wk), "wv8": pack3(Wv), "wq8": pack3(Wq),
        "ones8": np.ones((128, 2, 1), np.float32).astype(f8),
        "indsel": indsel.astype(f8c),
        "cbb": cbb, "cf": cf,
    }


_NC = None


def _build():
    nc = bacc.Bacc("TRN2", target_bir_lowering=False)
    ins = {}
    ins["xqb"] = nc.dram_tensor("xqb", [CIN, L], BF16, kind="ExternalInput").ap()
    ins["xq8"] = nc.dram_tensor("xq8", [CIN, L], FP8, kind="ExternalInput").ap()
    ins["xk8"] = nc.dram_tensor("xk8", [CIN, L], FP8, kind="ExternalInput").ap()
    ins["xv8"] = nc.dram_tensor("xv8", [CIN, L], FP8, kind="ExternalInput").ap()
    ins["wk8"] = nc.dram_tensor("wk8", [128, 2, 256], FP8, kind="ExternalInput").ap()
    ins["wv8"] = nc.dram_tensor("wv8", [128, 2, 256], FP8, kind="ExternalInput").ap()
    ins["wq8"] = nc.dram_tensor("wq8", [128, 2, 256], FP8, kind="ExternalInput").ap()
    ins["ones8"] = nc.dram_tensor("ones8", [128, 2, 1], FP8, kind="ExternalInput").ap()
    ins["indsel"] = nc.dram_tensor("indsel", [128, 4, 256], FP8, kind="ExternalInput").ap()
    ins["cbb"] = nc.dram_tensor("cbb", [128, CBB_COLS], BF16, kind="ExternalInput").ap()
    ins["cf"] = nc.dram_tensor("cf", [128, CF_COLS], F32, kind="ExternalInput").ap()
    out_ap = nc.dram_tensor("out", [CIN, L], BF16, kind="ExternalOutput").ap()
    with tile.TileContext(nc) as tc:
        _emit(tc, ins, out_ap)
    nc.compile()
    return nc


def get_nc():
    global _NC
    if _NC is None:
        _NC = _build()
    return _NC


def make_in_maps(inputs):
    bf = ml_dtypes.bfloat16
    f8 = ml_dtypes.float8_e4m3
    consts = _build_consts(
        inputs["Wq"], inputs["bq"], inputs["Wk"], inputs["bk"],
        inputs["Wv"], inputs["bv"], inputs["Wr"], inputs["br"],
    )
    qf32 = np.asarray(inputs["query_feature"], np.float32).reshape(N, CIN, L)
    qf = qf32.astype(bf)
    qf8 = qf32.astype(f8)
    kf = np.asarray(inputs["key_feature"], np.float32).reshape(N, CIN, L).astype(f8)
    vf = np.asarray(inputs["value_feature"], np.float32).reshape(N, CIN, L).astype(f8)
    return [
        {"xqb": np.ascontiguousarray(qf[i]),
         "xq8": np.ascontiguousarray(qf8[i]),
         "xk8": np.ascontiguousarray(kf[i]),
         "xv8": np.ascontiguousarray(vf[i]),
         **consts}
        for i in range(N_CORES)
    ]


def kernel(query_feature, key_feature, value_feature,
           Wq, bq, Wk, bk, Wv, bv, Wr, br):
    nc = get_nc()
    in_maps = make_in_maps(dict(
        query_feature=query_feature, key_feature=key_feature,
        value_feature=value_feature, Wq=Wq, bq=bq, Wk=Wk, bk=bk,
        Wv=Wv, bv=bv, Wr=Wr, br=br,
    ))
    res = run_bass_kernel_spmd(nc, in_maps, core_ids=list(range(N_CORES)))
    out = np.stack(
        [res.results[i]["out"].astype(np.float32) for i in range(N_CORES)]
    )
    return out.reshape(N, CIN, H_IMG, W_IMG)
